# revision 38
# baseline (speedup 1.0000x reference)
"""Trainium2 Bass kernel for the NumReps masked-mean problem.

Math: each mask row is a contiguous run of ones (1..8 long). expand_window
widens it by int(0.2*len) (== 1 iff len >= 5) on each side, clamped to
[0, S-1]; the output row is the mean of reps rows over the widened window
(window length n <= 10, and n is never 5 or 6).

Strategy (per core, data-parallel over batch: 16 batches / 8 cores = 2):
  - run length via scalar-engine accumulate, position-sum via one fused
    scalar_tensor_tensor pass over the mask -> first index recovered
    exactly (rint trick)
  - derive window start ns, length n, weight 1/n with tiny [128,1] ops
  - two indirect-DMA gathers per batch: rows ns..ns+3 always (one 16KB
    descriptor per output row), rows ns+4..ns+9 only for rows with
    n >= 7 (out-of-range index -> skipped; the landing area is
    pre-zeroed, contributing exact zeros)
  - weighted windowed sum on the TensorEngine: 10 accumulating diagonal
    matmuls in float32r (full rate at N=512), diag_j = diag((j<n)/n);
    the gathered chunks are staged through f32r tiles on DVE (the BIR
    verifier requires fp32r inputs to come from a rounding op)
  - PSUM -> SBUF copies (DVE + ACT), store
"""

import numpy as np

B, M, S, D = 16, 128, 2048, 1024
NCORES = 8
BPC = B // NCORES  # batches per core
WMAX = 10  # max expanded window length
G1 = 4  # chunks in the unconditional gather (n is 1..4 or 7..10; the
        # conditional gather covers chunks G1..9 for n >= 7 rows only)
RINT_MAGIC = 12582912.0  # 2^23 + 2^22: (x + magic) - magic == rint(x) for |x| < 2^22

# weighted-reduce dtype: "pe_f32r" (fast) | "pe_f32" (exact, slower PE)
REDUCE_MODE = "pe_f32r"

_cache = {}


def _build_nc():
    import concourse.bacc as bacc
    import concourse.bass as bass
    import concourse.mybir as mybir
    from concourse import tile

    f32 = mybir.dt.float32
    f32r = mybir.dt.float32r
    i32 = mybir.dt.int32
    Alu = mybir.AluOpType
    Act = mybir.ActivationFunctionType

    nc = bacc.Bacc("TRN2", target_bir_lowering=False, debug=False)

    mask = nc.dram_tensor("mask", [BPC, M, S], f32, kind="ExternalInput")
    reps = [
        nc.dram_tensor(f"reps{b}", [S, D], f32, kind="ExternalInput")
        for b in range(BPC)
    ]
    out = nc.dram_tensor("out", [BPC, M, D], f32, kind="ExternalOutput")

    iota_np = np.broadcast_to(np.arange(S, dtype=np.int16), (M, S))
    iota_const = nc.inline_tensor(np.ascontiguousarray(iota_np), name="iota_const")

    with tile.TileContext(nc) as tc:
        with (
            tc.tile_pool(name="const", bufs=1) as cpool,
            tc.tile_pool(name="big", bufs=2) as big,
            tc.tile_pool(name="small", bufs=2) as small,
            tc.tile_pool(name="psum", bufs=2, space="PSUM") as psum,
        ):
            # constants: iota row (DMA'd from an inline NEFF tensor) and a
            # 128x128 identity (gpsimd, idle early)
            iota_f = cpool.tile([M, S], mybir.dt.int16)
            nc.sync.dma_start(iota_f[:], iota_const[:])
            ident = cpool.tile([M, M], f32)
            nc.gpsimd.memset(ident[:], 1.0)
            nc.gpsimd.affine_select(
                out=ident[:], in_=ident[:], compare_op=Alu.is_equal,
                fill=0.0, base=0, pattern=[[-1, M]], channel_multiplier=1,
            )

            # explicit gather tiles (one per batch) so the conditional tail
            # can be pre-zeroed once, off the critical path
            gts = [
                cpool.tile([M, WMAX * D], f32, tag=f"gt{b}", name=f"gt{b}")
                for b in range(BPC)
            ]
            for b in range(BPC):
                nc.gpsimd.memset(gts[b][:, G1 * D:], 0.0)

            for b in range(BPC):
                gt = gts[b]
                mt = big.tile([M, S], f32, tag="mask")
                nc.sync.dma_start(mt[:], mask[b])

                # len on the scalar engine: accum_out = sum(mask); the
                # elementwise output lands in a stride-0 one-column sink
                lsink = small.tile([M, 1], f32, tag="lsink")
                lenf = small.tile([M, 1], f32, tag="lenf")
                lsink_ap = bass.AP(
                    lsink[:].tensor, lsink[:].offset, [lsink[:].ap[0], [0, S]]
                )
                nc.scalar.activation(
                    out=lsink_ap, in_=mt[:], func=Act.Identity,
                    accum_out=lenf[:],
                )
                # one DVE pass: A1 = sum((iota-4096)*mask) = possum - 4096*len
                ssink = small.tile([M, 1], f32, tag="ssink")
                a1 = small.tile([M, 1], f32, tag="a1")
                ssink_ap = bass.AP(
                    ssink[:].tensor, ssink[:].offset, [ssink[:].ap[0], [0, S]]
                )
                nc.vector.scalar_tensor_tensor(
                    out=ssink_ap, in0=iota_f[:], scalar=-4096.0, in1=mt[:],
                    op0=Alu.add, op1=Alu.mult, accum_out=a1[:],
                )
                # possum = A1 + 4096*len (exact)
                psm = small.tile([M, 1], f32, tag="psm")
                nc.vector.tensor_scalar(
                    out=psm[:], in0=lenf[:], scalar1=4096.0,
                    scalar2=a1[:, :1], op0=Alu.mult, op1=Alu.add,
                )

                # first = rint(possum/len - (len-1)/2)
                rl = small.tile([M, 1], f32, tag="rl")
                nc.vector.reciprocal(rl[:], lenf[:])
                half_lm1 = small.tile([M, 1], f32, tag="hlm1")
                nc.vector.tensor_scalar(
                    out=half_lm1[:], in0=lenf[:], scalar1=-1.0, scalar2=0.5,
                    op0=Alu.add, op1=Alu.mult,
                )
                first = small.tile([M, 1], f32, tag="first")
                nc.vector.tensor_scalar(
                    out=first[:], in0=psm[:], scalar1=rl[:, :1],
                    scalar2=half_lm1[:, :1], op0=Alu.mult, op1=Alu.subtract,
                )
                nc.vector.tensor_scalar(
                    out=first[:], in0=first[:], scalar1=RINT_MAGIC,
                    scalar2=-RINT_MAGIC, op0=Alu.add, op1=Alu.add,
                )
                last = small.tile([M, 1], f32, tag="last")
                nc.vector.tensor_scalar(
                    out=last[:], in0=first[:], scalar1=lenf[:, :1],
                    scalar2=-1.0, op0=Alu.add, op1=Alu.add,
                )

                # expand = 1 iff len >= 5
                e = small.tile([M, 1], f32, tag="e")
                nc.vector.tensor_scalar(
                    out=e[:], in0=lenf[:], scalar1=4.5, scalar2=None,
                    op0=Alu.is_ge,
                )
                # ns = max(first-e, 0); ne = min(last+e, S-1); n = ne-ns+1
                ns = small.tile([M, 1], f32, tag="ns")
                nc.vector.tensor_scalar(
                    out=ns[:], in0=first[:], scalar1=e[:, :1], scalar2=0.0,
                    op0=Alu.subtract, op1=Alu.max,
                )
                ne = small.tile([M, 1], f32, tag="ne")
                nc.vector.tensor_scalar(
                    out=ne[:], in0=last[:], scalar1=e[:, :1],
                    scalar2=float(S - 1), op0=Alu.add, op1=Alu.min,
                )
                n = small.tile([M, 1], f32, tag="n")
                nc.vector.tensor_scalar(
                    out=n[:], in0=ne[:], scalar1=ns[:, :1], scalar2=1.0,
                    op0=Alu.subtract, op1=Alu.add,
                )
                inv = small.tile([M, 1], f32, tag="inv")
                nc.vector.reciprocal(inv[:], n[:])

                nsi = small.tile([M, 1], i32, tag="nsi")
                nc.vector.tensor_copy(nsi[:], ns[:])
                # second gather index: ns+G1, pushed out of range (skipped)
                # for rows with n < 7 (e == 0)
                idx2 = small.tile([M, 1], f32, tag="idx2")
                nc.vector.tensor_scalar(
                    out=idx2[:], in0=e[:], scalar1=-4096.0,
                    scalar2=ns[:, :1], op0=Alu.mult, op1=Alu.add,
                )
                nc.vector.tensor_scalar_add(idx2[:], idx2[:], 4096.0 + G1)
                nsi2 = small.tile([M, 1], i32, tag="nsi2")
                nc.vector.tensor_copy(nsi2[:], idx2[:])

                # weights: w[m, j] = (j < n_m) * inv_m     [M, WMAX]
                w = small.tile([M, WMAX], f32, tag="w")
                nc.vector.tensor_scalar(
                    out=w[:], in0=iota_f[:, :WMAX], scalar1=n[:, :1],
                    scalar2=inv[:, :1], op0=Alu.is_lt, op1=Alu.mult,
                )

                # gathers: head chunks always, tail only where n >= 7
                nc.gpsimd.indirect_dma_start(
                    out=gt[:, :G1 * D],
                    out_offset=None,
                    in_=reps[b][:],
                    in_offset=bass.IndirectOffsetOnAxis(ap=nsi[:, :1], axis=0),
                )
                nc.gpsimd.indirect_dma_start(
                    out=gt[:, G1 * D:],
                    out_offset=None,
                    in_=reps[b][:],
                    in_offset=bass.IndirectOffsetOnAxis(ap=nsi2[:, :1], axis=0),
                    bounds_check=S - 1,
                    oob_is_err=False,
                )

                # diag_j = diag(w[:, j]): identity * broadcast weight on DVE
                # (f32r output doubles as the verifier-required rounding op)
                diag_r = big.tile([M, WMAX * M], f32r, tag="diag_r")
                nc.vector.tensor_tensor(
                    out=diag_r[:].rearrange("p (j q) -> p j q", j=WMAX),
                    in0=ident[:].unsqueeze(1).to_broadcast([M, WMAX, M]),
                    in1=w[:].unsqueeze(-1).to_broadcast([M, WMAX, M]),
                    op=Alu.mult,
                )

                osum = big.tile([M, D], f32, tag="osum")
                ps0 = psum.tile([M, 512], f32, tag="ps0")
                ps1 = psum.tile([M, 512], f32, tag="ps1")
                # fp32r rhs must be produced by a rounding op (the verifier
                # keys on the memory location, so the DMA-written gather tile
                # can't feed the PE directly): stage pairs of chunks through
                # f32r tiles on DVE
                gtrs = []
                for h in range(WMAX // 2):
                    gtr = big.tile([M, 2 * D], f32r, tag="gtr", bufs=3,
                                   name=f"gtr_{b}_{h}")
                    nc.vector.tensor_copy(gtr[:], gt[:, 2 * h * D:(2 * h + 2) * D])
                    gtrs.append(gtr)
                for j in range(WMAX):
                    dj = diag_r[:, j * M:(j + 1) * M]
                    seg = gtrs[j // 2][:, (j % 2) * D:(j % 2 + 1) * D]
                    nc.tensor.matmul(
                        ps0[:], lhsT=dj, rhs=seg[:, :512],
                        start=(j == 0), stop=(j == WMAX - 1),
                    )
                    nc.tensor.matmul(
                        ps1[:], lhsT=dj, rhs=seg[:, 512:],
                        start=(j == 0), stop=(j == WMAX - 1),
                    )
                nc.vector.tensor_copy(osum[:, :512], ps0[:])
                nc.scalar.copy(osum[:, 512:], ps1[:])
                nc.sync.dma_start(out[b], osum[:])

    nc.finalize()
    return nc


def _get_nc():
    if "nc" not in _cache:
        _cache["nc"] = _build_nc()
    return _cache["nc"]


def _shard_inputs(number_mask, reps):
    in_maps = []
    for c in range(NCORES):
        m = {"mask": np.ascontiguousarray(number_mask[c * BPC:(c + 1) * BPC])}
        for b in range(BPC):
            m[f"reps{b}"] = np.ascontiguousarray(reps[c * BPC + b])
        in_maps.append(m)
    return in_maps


def _install_ntff_hook():
    """The image's antenv lacks axon_hooks; synthesize it so trace=True
    (NTFF profiling) works through run_bass_kernel_spmd."""
    import sys
    import types

    try:
        from antenv.axon_hooks import get_axon_ntff_profile_hook  # noqa: F401
        return
    except ImportError:
        pass
    from trn_agent_boot.trn_boot import _ntff_profile_via_ctypes

    mod = types.ModuleType("antenv.axon_hooks")
    _hook = [_ntff_profile_via_ctypes("/opt/axon/libaxon_pjrt.so")]
    mod.get_axon_ntff_profile_hook = lambda: _hook[0]
    mod.set_axon_ntff_profile_hook = lambda h: _hook.__setitem__(0, h)
    sys.modules["antenv.axon_hooks"] = mod
    import antenv

    antenv.axon_hooks = mod


def _run(number_mask, reps, trace=False):
    from concourse.bass_utils import run_bass_kernel_spmd

    if trace:
        _install_ntff_hook()
    nc = _get_nc()
    in_maps = _shard_inputs(number_mask, reps)
    res = run_bass_kernel_spmd(
        nc, in_maps, core_ids=list(range(NCORES)), trace=trace
    )
    outs = np.stack([r["out"] for r in res.results], axis=0)
    return outs.reshape(B, M, D), res


def kernel(**inputs):
    out, _ = _run(inputs["number_mask"], inputs["reps"], trace=False)
    return out
